# revision 15
# baseline (speedup 1.0000x reference)
# MoE (15 routed experts, top-2, + shared expert) on 8 trn2 NeuronCores.
#
# Strategy: all routing runs on the HOST (fp64 gate -> top-2 -> per-expert
# token lists -> packed dense inputs); the device kernel is a pure static
# dense-FFN pipeline, which keeps the PE streaming with zero serial
# dispatch chain. Expert-parallel sharding: 16 slots across 8 cores
# (slot0 capacity CAP0=596 for the 8 largest whole experts, slot1
# capacity CAP1=544 for the rest; an expert larger than CAP0 is split
# into pieces). The shared expert is data-parallel (512 tokens/core).
# Combine weights (top-2 softmax probs) are applied on the host during
# the scatter-add, so the device computes plain SwiGLU FFNs; any token
# overflowing the static capacities falls back to numpy on the host.
#
# Matmul structure per slot: h-phase keeps W1/W3 tiles stationary and
# streams all N tokens per weight load; y-phase keeps 128-token h tiles
# stationary and streams the full 2048-wide W2^T per load (2048 columns
# per LDWEIGHTS). Redundant LDWEIGHTS are deduped by rewriting the
# serialized BIR before walrus codegen (_dedup_ldw): the tile scheduler
# emits one Ldweights per matmul even for same-weight runs, and each
# redundant reload would force a PE pipeline drain (~270ns). DMA is
# ordered for overlap: the w1/w3 stream owns the sync DGE ring, while x
# staging, w2^T prefetch, and y writebacks ride the scalar ring; slot0's
# first x/weight tiles are sliced into 4-dk chunks so the first matmul
# fires ~13us into the kernel.
import numpy as np
import ml_dtypes

DIM = 2048
INTER = 1408
NE = 15
TOPK = 2
T = 4096
NCORES = 8
TSH = T // NCORES     # shared-expert tokens per core
DI = DIM // 128       # 16 contraction tiles over d
II = INTER // 128     # 11 tiles over inter dim
CAP0 = 596            # slot0 token capacity (largest expert load)
CAP1 = 544            # slot1 token capacity

_PROG = {}
_STATE = {}


def _nblocks(n):
    # PSUM-bank column blocks covering n columns (<=512 each)
    out = []
    o = 0
    while o < n:
        out.append((o, min(512, n - o)))
        o += 512
    return out


def _ttiles(n):
    # 128-token stationary tiles covering n tokens
    out = []
    o = 0
    while o < n:
        out.append((o, min(128, n - o)))
        o += 128
    return out


def _dedup_ldw(d):
    """Drop redundant PE Ldweights (same weights AP as the currently loaded
    one) from a serialized BIR module. The tile scheduler emits one
    Ldweights per matmul even when consecutive matmuls share the stationary
    operand; each redundant reload forces a pipeline drain + reload
    (~270ns). Waits on a dropped Ldweights that are not dominated by an
    earlier wait in the same engine FIFO are preserved by converting the
    instruction to an EventSemaphore instead of deleting it."""
    import json as _json
    removed = 0
    for fn in d.get("functions", []):
        for blk in fn.get("blocks", []):
            insts = blk.get("instructions", [])
            out = []
            cur_w = None
            waited = {}

            def track(inst):
                for w in (inst.get("sync_info") or {}).get("on_wait", []):
                    if w.get("wait_mode") == "sem-ge-imm":
                        k = (w.get("id"), w.get("ant_name"))
                        v = w.get("wait_value", 0)
                        if v > waited.get(k, -1):
                            waited[k] = v

            for inst in insts:
                if inst.get("engine") != "PE":
                    out.append(inst)
                    continue
                op = inst.get("opcode")
                if op == "Ldweights":
                    sig = _json.dumps(
                        [inst.get("ins"), inst.get("tile_position"),
                         inst.get("tile_size"), inst.get("perf_mode"),
                         inst.get("is_transpose")], sort_keys=True)
                    if sig == cur_w:
                        si = inst.get("sync_info") or {}
                        keep = [w for w in si.get("on_wait", [])
                                if not (w.get("wait_mode") == "sem-ge-imm"
                                        and waited.get(
                                            (w.get("id"), w.get("ant_name")),
                                            -1) >= w.get("wait_value", 0))]
                        ups = si.get("on_update", [])
                        if keep or ups:
                            ev = {"opcode": "EventSemaphore",
                                  "engine": "PE",
                                  "name": inst["name"],
                                  "debug": inst.get("debug"),
                                  "ins": [], "outs": [],
                                  "sync_info": {"on_wait": keep,
                                                "on_update": ups}}
                            track(ev)
                            out.append(ev)
                        removed += 1
                        continue
                    cur_w = sig
                    track(inst)
                    out.append(inst)
                elif op in ("Matmult", "EventSemaphore"):
                    track(inst)
                    out.append(inst)
                else:
                    cur_w = None
                    track(inst)
                    out.append(inst)
            blk["instructions"] = out
    return removed


def _patch_ldw_dedup():
    import concourse.bass_utils as BU
    if getattr(BU, "_ldw_dedup", False):
        return
    orig = BU.bir_verify_and_optimise

    def patched(tmpdir, inp="bir.json", *args, **kw):
        import os
        import json as _json
        try:
            p = os.path.join(str(tmpdir), inp)
            with open(p) as f:
                d = _json.load(f)
            n = _dedup_ldw(d)
            if n:
                with open(p, "w") as f:
                    _json.dump(d, f)
            _STATE["ldw_removed"] = n
        except Exception as e:  # fall back to unmodified BIR
            _STATE["ldw_dedup_error"] = repr(e)
        return orig(tmpdir, inp, *args, **kw)

    BU.bir_verify_and_optimise = patched
    BU._ldw_dedup = True


def build_program():
    if "nc" in _PROG:
        return _PROG["nc"]
    from contextlib import ExitStack
    import concourse.bacc as bacc
    import concourse.mybir as mybir
    import concourse.tile as tile

    _patch_ldw_dedup()

    fp32 = mybir.dt.float32
    bf16 = mybir.dt.bfloat16
    AF = mybir.ActivationFunctionType

    nc = bacc.Bacc("TRN2", target_bir_lowering=False, debug=False,
                   num_devices=NCORES)

    # ---- I/O ----
    xts = []
    w13s = []
    w2ts = []
    youts = []
    for s, cap in ((0, CAP0), (1, CAP1), (2, TSH)):
        xts.append(nc.dram_tensor(f"xt{s}", [128, DI, cap], bf16,
                                  kind="ExternalInput").ap())
        w13s.append(nc.dram_tensor(f"w13_{s}", [II, 2, 128, DI, 128], bf16,
                                   kind="ExternalInput").ap())
        w2ts.append(nc.dram_tensor(f"w2t_{s}", [II, 128, DIM], bf16,
                                   kind="ExternalInput").ap())
        youts.append(nc.dram_tensor(f"y{s}", [cap, DIM], fp32,
                                    kind="ExternalOutput").ap())

    with tile.TileContext(nc) as tc, ExitStack() as ctx:
        xpool = ctx.enter_context(tc.tile_pool(name="xpool", bufs=1))
        wpool = ctx.enter_context(tc.tile_pool(name="wpool", bufs=3))
        w2pool = ctx.enter_context(tc.tile_pool(name="w2pool", bufs=13))
        hpool = ctx.enter_context(tc.tile_pool(name="hpool", bufs=2))
        spool = ctx.enter_context(tc.tile_pool(name="spool", bufs=2))
        ypool = ctx.enter_context(tc.tile_pool(name="ypool", bufs=2))
        psp = ctx.enter_context(tc.tile_pool(name="psp", bufs=4,
                                             space="PSUM"))

        # x tiles are DMA'd in 4-dk slices so the first matmuls only wait
        # on the first slice, not the whole 2.5MB stage.
        xt_sb = []
        for s, cap in ((0, CAP0), (1, CAP1), (2, TSH)):
            xsb = xpool.tile([128, DI, cap], bf16, tag=f"xt{s}",
                             name=f"xt_sb{s}")
            xt_sb.append(xsb)

        def stage_x(s, eng):
            for g in range(0, DI, 4):
                eng.dma_start(out=xt_sb[s][:, g:g + 4, :],
                              in_=xts[s][:, g:g + 4, :])

        # Slot order: shared first (its input is staged at startup), the
        # 32-token tail of slot1 lands last so the exit sequence waits on
        # the smallest possible final tile.
        order = [(2, TSH), (0, CAP0), (1, CAP1)]

        # startup: the first slot's x rides the scalar ring so it transfers
        # in parallel with the first weight chunks on the sync ring (DMA
        # triggers cost ~0.7us each and each ring is FIFO, so putting both
        # on one ring would serialize them).
        s0 = order[0][0]
        nc.scalar.dma_start(out=xt_sb[s0][:, 0:4, :], in_=xts[s0][:, 0:4, :])

        for idx, (s, cap) in enumerate(order):
            xsb = xt_sb[s]
            nb = _nblocks(cap)
            tt = _ttiles(cap)
            w2sb = [w2pool.tile([128, DIM], bf16, tag="w2", name="w2b")
                    for ib in range(II)]

            # ---- h-phase: W1/W3 stationary, tokens streaming ----
            hT = hpool.tile([128, II, cap], bf16, tag="hT", name="hT")
            for it in range(II):
                w1b = wpool.tile([128, DI, 128], bf16, tag="w1b", name="w1b")
                w3b = wpool.tile([128, DI, 128], bf16, tag="w3b", name="w3b")
                if idx == 0 and it == 0:
                    # chunked weight loads so dk-group g's matmuls only
                    # wait on chunk g; the remaining x slices go on the
                    # (otherwise idle) scalar ring in parallel.
                    for g in range(4, DI, 4):
                        nc.scalar.dma_start(out=xsb[:, g:g + 4, :],
                                            in_=xts[s][:, g:g + 4, :])
                    for g in range(0, DI, 4):
                        nc.sync.dma_start(out=w1b[:, g:g + 4, :],
                                          in_=w13s[s][it, 0][:, g:g + 4, :])
                        nc.sync.dma_start(out=w3b[:, g:g + 4, :],
                                          in_=w13s[s][it, 1][:, g:g + 4, :])
                else:
                    nc.sync.dma_start(out=w1b, in_=w13s[s][it, 0])
                    nc.sync.dma_start(out=w3b, in_=w13s[s][it, 1])
                if it == 4:
                    # w2^T prefetch on the scalar (ACT) DGE ring, deferred
                    # past the early weight tiles so it doesn't compete
                    # with the critical h-phase weight stream.
                    for ib in range(II):
                        nc.scalar.dma_start(out=w2sb[ib], in_=w2ts[s][ib])
                if it == 8 and idx < 2:
                    stage_x(order[idx + 1][0], nc.scalar)
                ph1 = psp.tile([128, 2, 512], fp32, tag="ps", name="ph1")
                ph3 = psp.tile([128, 2, 512], fp32, tag="ps", name="ph3")
                for dk in range(DI):
                    st = dk == 0
                    sp = dk == DI - 1
                    for b, (n0, nn) in enumerate(nb):
                        nc.tensor.matmul(ph1[:, b, :nn], lhsT=w1b[:, dk, :],
                                         rhs=xsb[:, dk, n0:n0 + nn],
                                         start=st, stop=sp)
                    for b, (n0, nn) in enumerate(nb):
                        nc.tensor.matmul(ph3[:, b, :nn], lhsT=w3b[:, dk, :],
                                         rhs=xsb[:, dk, n0:n0 + nn],
                                         start=st, stop=sp)
                s1 = spool.tile([128, cap], fp32, tag="s1", name="s1")
                for b, (n0, nn) in enumerate(nb):
                    nc.scalar.activation(s1[:, n0:n0 + nn], ph1[:, b, :nn],
                                         AF.Silu)
                    nc.vector.tensor_mul(hT[:, it, n0:n0 + nn],
                                         s1[:, n0:n0 + nn], ph3[:, b, :nn])

            # ---- y-phase: h tiles stationary, W2^T streaming ----
            for t0, tn in tt:
                ya = psp.tile([128, 2, 512], fp32, tag="ps", name="ya")
                yb = psp.tile([128, 2, 512], fp32, tag="ps", name="yb")
                for ib in range(II):
                    st = ib == 0
                    sp = ib == II - 1
                    lhs = hT[:, ib, t0:t0 + tn]
                    nc.tensor.matmul(ya[:tn, 0, :], lhsT=lhs,
                                     rhs=w2sb[ib][:, 0:512],
                                     start=st, stop=sp)
                    nc.tensor.matmul(ya[:tn, 1, :], lhsT=lhs,
                                     rhs=w2sb[ib][:, 512:1024],
                                     start=st, stop=sp)
                    nc.tensor.matmul(yb[:tn, 0, :], lhsT=lhs,
                                     rhs=w2sb[ib][:, 1024:1536],
                                     start=st, stop=sp)
                    nc.tensor.matmul(yb[:tn, 1, :], lhsT=lhs,
                                     rhs=w2sb[ib][:, 1536:2048],
                                     start=st, stop=sp)
                # last slot's writebacks use the sync ring (no weight
                # stream left there), draining the tail in parallel with
                # the scalar ring's remaining transfers.
                oeng = nc.sync if idx == 2 else nc.scalar
                ysb = ypool.tile([128, 4, 512], fp32, tag="ysb", name="ysb")
                nc.scalar.copy(ysb[:tn, 0, :], ya[:tn, 0, :])
                nc.vector.tensor_copy(ysb[:tn, 1, :], ya[:tn, 1, :])
                oeng.dma_start(
                    out=youts[s][t0:t0 + tn, 0:1024],
                    in_=ysb[:tn, 0:2].rearrange("p a b -> p (a b)"))
                nc.scalar.copy(ysb[:tn, 2, :], yb[:tn, 0, :])
                nc.vector.tensor_copy(ysb[:tn, 3, :], yb[:tn, 1, :])
                oeng.dma_start(
                    out=youts[s][t0:t0 + tn, 1024:2048],
                    in_=ysb[:tn, 2:4].rearrange("p a b -> p (a b)"))

    nc.compile()
    _PROG["nc"] = nc
    return nc


def _route(xf, gate_w):
    # fp64 gate: softmax over routed experts, top-2 (matches fp32 ref
    # ordering -- min top2/top3 logit gap >> fp64 matmul error)
    logits = xf.astype(np.float64) @ np.asarray(gate_w, np.float64).T
    p = np.exp(logits - logits.max(-1, keepdims=True))
    p /= p.sum(-1, keepdims=True)
    idx = np.argsort(-p, axis=-1)[:, :TOPK]          # [T, 2]
    wts = np.take_along_axis(p, idx, axis=-1)        # [T, 2]
    return idx.astype(np.int64), wts.astype(np.float32)


def _make_slots(idx, wts):
    """Assign (expert, token-list, weight-list) to 16 slots: 8 of CAP0,
    8 of CAP1. Returns (slots0, slots1, leftovers); each slot is
    (expert, tokens, weights); leftovers is a list of the same for
    tokens that did not fit (numpy fallback)."""
    ntok = idx.shape[0]
    per_e_tok = [[] for _ in range(NE)]
    per_e_w = [[] for _ in range(NE)]
    flat_t = np.repeat(np.arange(ntok), TOPK)
    flat_e = idx.reshape(-1)
    flat_w = wts.reshape(-1)
    order = np.argsort(flat_e, kind="stable")
    for e, t, w in zip(flat_e[order], flat_t[order], flat_w[order]):
        per_e_tok[e].append(t)
        per_e_w[e].append(w)

    items = []  # (count, expert, tokens, weights)
    for e in range(NE):
        toks = np.array(per_e_tok[e], np.int64)
        ws = np.array(per_e_w[e], np.float32)
        if len(toks) > CAP0:
            nparts = -(-len(toks) // CAP1)
            for part in range(nparts):
                sl = slice(part * len(toks) // nparts,
                           (part + 1) * len(toks) // nparts)
                items.append((len(toks[sl]), e, toks[sl], ws[sl]))
        else:
            items.append((len(toks), e, toks, ws))
    items.sort(key=lambda x: -x[0])

    slots0, slots1, leftovers = [], [], []
    for cnt, e, toks, ws in items:
        if len(slots0) < 8 and cnt <= CAP0 and (cnt > CAP1 or
                                                len(items) - len(slots1) <= 16 - len(slots0)):
            slots0.append((e, toks[:CAP0], ws[:CAP0]))
            if cnt > CAP0:
                leftovers.append((e, toks[CAP0:], ws[CAP0:]))
        elif len(slots1) < 8:
            slots1.append((e, toks[:CAP1], ws[:CAP1]))
            if cnt > CAP1:
                leftovers.append((e, toks[CAP1:], ws[CAP1:]))
        else:
            leftovers.append((e, toks, ws))
    while len(slots0) < 8:
        slots0.append((0, np.zeros(0, np.int64), np.zeros(0, np.float32)))
    while len(slots1) < 8:
        slots1.append((0, np.zeros(0, np.int64), np.zeros(0, np.float32)))
    return slots0, slots1, leftovers


def _pack13(w1e, w3e, bf):
    # [INTER, DIM] x2 -> [II, 2, 128, DI, 128] stationary lhsT tiles
    out = np.empty((II, 2, 128, DI, 128), bf)
    for m, w in ((0, w1e), (1, w3e)):
        out[:, m] = np.asarray(w, np.float32).reshape(
            II, 128, DI, 128).transpose(0, 3, 2, 1).astype(bf)
    return np.ascontiguousarray(out)


def _pack2(w2e, bf):
    # [DIM, INTER] -> [II, 128, DIM] moving w2^T tiles
    return np.ascontiguousarray(
        np.asarray(w2e, np.float32).T.reshape(II, 128, DIM)).astype(bf)


def _packx(xf_rows, cap, bf):
    # [n, DIM] fp32 -> [128, DI, cap] bf16 (zero-padded)
    n = xf_rows.shape[0]
    out = np.zeros((128, DI, cap), bf)
    if n:
        out[:, :, :n] = xf_rows.T.reshape(DI, 128, n).transpose(1, 0, 2).astype(bf)
    return out


def prep_in_maps(x, gate_w, w1, w2, w3, sw1, sw2, sw3):
    bf = ml_dtypes.bfloat16
    xf = np.ascontiguousarray(np.asarray(x, np.float32).reshape(-1, DIM))
    ntok = xf.shape[0]
    assert ntok == T and xf.shape[1] == DIM

    idx, wts = _route(xf, gate_w)
    slots0, slots1, leftovers = _make_slots(idx, wts)
    _STATE["slots0"] = slots0
    _STATE["slots1"] = slots1
    _STATE["leftovers"] = leftovers
    _STATE["inputs"] = (xf, w1, w2, w3)

    pack13_cache = {}
    pack2_cache = {}

    def get13(e):
        if e not in pack13_cache:
            pack13_cache[e] = _pack13(w1[e], w3[e], bf)
        return pack13_cache[e]

    def get2(e):
        if e not in pack2_cache:
            pack2_cache[e] = _pack2(w2[e], bf)
        return pack2_cache[e]

    sh13 = _pack13(sw1, sw3, bf)
    sh2 = _pack2(sw2, bf)

    in_maps = []
    for c in range(NCORES):
        e0, t0, _ = slots0[c]
        e1, t1, _ = slots1[c]
        in_maps.append({
            "xt0": _packx(xf[t0], CAP0, bf),
            "xt1": _packx(xf[t1], CAP1, bf),
            "xt2": _packx(xf[c * TSH:(c + 1) * TSH], TSH, bf),
            "w13_0": get13(e0), "w2t_0": get2(e0),
            "w13_1": get13(e1), "w2t_1": get2(e1),
            "w13_2": sh13, "w2t_2": sh2,
        })
    return in_maps


def assemble(results, out_shape):
    y = np.zeros((T, DIM), np.float32)
    slots0, slots1 = _STATE["slots0"], _STATE["slots1"]
    for c in range(NCORES):
        r = results[c]
        for slots, key in ((slots0, "y0"), (slots1, "y1")):
            _, toks, ws = slots[c]
            n = len(toks)
            if n:
                blk = np.asarray(r[key])[:n]
                np.add.at(y, toks, blk * ws[:, None])
        y[c * TSH:(c + 1) * TSH] += np.asarray(r["y2"])
    # numpy fallback for any tokens that did not fit the static capacities
    leftovers = _STATE["leftovers"]
    if leftovers:
        xf, w1, w2, w3 = _STATE["inputs"]
        for e, toks, ws in leftovers:
            if len(toks) == 0:
                continue
            xe = xf[toks]
            h1 = xe @ np.asarray(w1[e], np.float32).T
            h3 = xe @ np.asarray(w3[e], np.float32).T
            h = (h1 / (1 + np.exp(-h1))) * h3
            y[toks] += (h @ np.asarray(w2[e], np.float32).T) * ws[:, None]
    return y.reshape(out_shape)


def run_on_hw(in_maps, trace=False, tmpdir=None):
    from concourse.bass_utils import run_bass_kernel_spmd
    nc = build_program()
    return run_bass_kernel_spmd(nc, in_maps, list(range(NCORES)),
                                trace=trace, tmpdir=tmpdir)


def kernel(x, gate_w, w1, w2, w3, sw1, sw2, sw3):
    in_maps = prep_in_maps(x, gate_w, w1, w2, w3, sw1, sw2, sw3)
    br = run_on_hw(in_maps)
    return assemble(br.results, np.asarray(x).shape)
